# revision 7
# baseline (speedup 1.0000x reference)
"""Trainium2 Bass kernel for 16-head MHA (B=4, L=2048, D=1024, fp32).

Sharding: batch x head-group over 8 cores. Core c handles batch c//2 and
heads (c%2)*8 .. (c%2)*8+7 (Megatron column-parallel QKV, row-parallel Wo).
Each core computes a partial output projection; the host sums the two
partials per batch.

Per-core device program (SPMD, identical on all cores):
  A1: VH = v @ Wv_g.T            -> [2048, 8*65] (65th col of each head = 1.0
                                    so the AV matmul also yields the softmax
                                    denominator as an extra output row)
  A2: QHT = Wq_g @ q.T, KHT = Wk_g @ k.T  -> [512, 2048] (head-dim major)
  B:  per head pair, per 1024-wide query block:
        scoresT[lk,:] = kh @ qh.T  (PE, fp32r, 2 heads packed in row tiles)
        E = exp(scoresT/8)         (ACT, PSUM->SBUF)
        O_unT += vh_aug.T @ E      (PE, accumulate over lk; row 64 = denom)
      normalize: O = O_un * (1/denom) broadcast via SBUF->SBUF DMA
  C:  outT = Wo_g.T @ OHT  (partial, summed on host)

All matmul operands are float32r (full-rate fp32 mode on the PE array,
~1e-4 relative error at K=1024).
"""

import sys

if "/opt/trn_rl_repo" not in sys.path:
    sys.path.insert(0, "/opt/trn_rl_repo")

import numpy as np

B, LQ, LV, D, H = 4, 2048, 2048, 1024, 16
DH = D // H            # 64
N_CORES = 8
H_LOC = H // 2         # 8 heads per core
HD_LOC = H_LOC * DH    # 512 head-dims per core
NKC = D // 128         # 8 contraction chunks for projections
NS = LV // 128         # 16 key chunks
NMC = HD_LOC // 128    # 4 head-dim chunks (head pairs)
JQ = 1024              # query block width in attention
VW = DH + 1            # 65: per-head V width incl. ones column

_PROG_CACHE = {}


def build_program(iters=1):
    import concourse.tile as tile
    from concourse import bacc, mybir

    F32 = mybir.dt.float32
    F32R = mybir.dt.float32r
    EXP = mybir.ActivationFunctionType.Exp

    nc = bacc.Bacc("TRN2", target_bir_lowering=False, debug=False,
                   num_devices=N_CORES)

    qT = nc.dram_tensor("qT", [D, LQ], F32R, kind="ExternalInput").ap()
    kT = nc.dram_tensor("kT", [D, LV], F32R, kind="ExternalInput").ap()
    vT = nc.dram_tensor("vT", [D, LV], F32R, kind="ExternalInput").ap()
    wqT = nc.dram_tensor("wqT", [D, HD_LOC], F32R, kind="ExternalInput").ap()
    wkT = nc.dram_tensor("wkT", [D, HD_LOC], F32R, kind="ExternalInput").ap()
    wvT = nc.dram_tensor("wvT", [D, HD_LOC], F32R, kind="ExternalInput").ap()
    woT = nc.dram_tensor("woT", [HD_LOC, D], F32R, kind="ExternalInput").ap()
    outT = nc.dram_tensor("outT", [D, LQ], F32, kind="ExternalOutput").ap()
    # DRAM bounce rows for broadcasting softmax 1/denom across partitions
    dbc = nc.dram_tensor("dbc", [2 * NMC * (LQ // JQ), JQ], F32).ap()

    with tile.TileContext(nc) as tc:
        with tc.tile_pool(name="persist", bufs=1) as persist:

            def body():
                # persistent activation tensors (live across phases)
                qht = [persist.tile([128, LQ], F32R, tag=f"qht{m}", name=f"qht{m}")
                       for m in range(NMC)]
                kht = [persist.tile([128, LV], F32R, tag=f"kht{m}", name=f"kht{m}")
                       for m in range(NMC)]
                vh = [persist.tile([128, H_LOC * VW], F32R, tag=f"vh{s}", name=f"vh{s}")
                      for s in range(NS)]
                oht = [persist.tile([128, LQ], F32R, tag=f"oht{m}", name=f"oht{m}")
                       for m in range(NMC)]
                ones8 = persist.tile([128, H_LOC], F32, tag="ones8", name="ones8")
                nc.vector.memset(ones8[:], 1.0)

                # ================= phase A: projections =================
                with (
                    tc.tile_pool(name="wproj", bufs=1) as wpool,
                    tc.tile_pool(name="act", bufs=18) as actp,
                    tc.tile_pool(name="psA", bufs=2, space="PSUM") as psA,
                ):
                    # ---- A1: V projection -> vh (with ones columns) ----
                    wv = []
                    for kc in range(NKC):
                        w = wpool.tile([128, HD_LOC], F32R, tag=f"w{kc}", name=f"wv{kc}")
                        nc.sync.dma_start(w[:], wvT[kc * 128:(kc + 1) * 128, :])
                        wv.append(w)
                    for sg in range(NS // 4):
                        vts = []
                        for kc in range(NKC):
                            t = actp.tile([128, 512], F32R, tag="act", name=f"vt{sg}_{kc}")
                            nc.sync.dma_start(
                                t[:], vT[kc * 128:(kc + 1) * 128,
                                         sg * 512:(sg + 1) * 512])
                            vts.append(t)
                        for si in range(4):
                            s = sg * 4 + si
                            p = psA.tile([128, HD_LOC], F32, tag="psA", name=f"pv{s}")
                            for kc in range(NKC):
                                nc.tensor.matmul(
                                    p[:], vts[kc][:, si * 128:(si + 1) * 128],
                                    wv[kc][:], start=(kc == 0), stop=(kc == NKC - 1))
                            v3 = vh[s].rearrange("p (h e) -> p h e", e=VW)
                            nc.vector.tensor_copy(
                                out=v3[:, :, DH:VW],
                                in_=ones8.rearrange("p (h o) -> p h o", o=1))
                            nc.vector.tensor_copy(
                                out=v3[:, :, 0:DH],
                                in_=p.rearrange("p (h e) -> p h e", e=DH))

                    # ---- A2: Q and K projections -> qht, kht ----
                    def project(dst, w_dram, x_dram, pfx):
                        ws = []
                        for kc in range(NKC):
                            w = wpool.tile([128, HD_LOC], F32R, tag=f"w{kc}",
                                           name=f"{pfx}w{kc}")
                            nc.sync.dma_start(w[:], w_dram[kc * 128:(kc + 1) * 128, :])
                            ws.append(w)
                        for n in range(LQ // 512):
                            xs = []
                            for kc in range(NKC):
                                t = actp.tile([128, 512], F32R, tag="act",
                                              name=f"{pfx}x{n}_{kc}")
                                nc.sync.dma_start(
                                    t[:], x_dram[kc * 128:(kc + 1) * 128,
                                                 n * 512:(n + 1) * 512])
                                xs.append(t)
                            for m in range(NMC):
                                p = psA.tile([128, 512], F32, tag="psA",
                                             name=f"{pfx}p{n}_{m}")
                                for kc in range(NKC):
                                    nc.tensor.matmul(
                                        p[:], ws[kc][:, m * 128:(m + 1) * 128],
                                        xs[kc][:],
                                        start=(kc == 0), stop=(kc == NKC - 1))
                                nc.vector.tensor_copy(
                                    out=dst[m][:, n * 512:(n + 1) * 512], in_=p[:])

                    project(qht, wqT, qT, "q")
                    project(kht, wkT, kT, "k")

                # ================= phase B: attention =================
                with (
                    tc.tile_pool(name="e", bufs=6) as epool,
                    tc.tile_pool(name="smalls", bufs=2) as smalls,
                    tc.tile_pool(name="rbc", bufs=2) as rbcp,
                    tc.tile_pool(name="pss", bufs=2, space="PSUM") as pss,
                    tc.tile_pool(name="pso", bufs=2, space="PSUM") as pso,
                ):
                    for c in range(NMC):
                        hA, hB = 2 * c, 2 * c + 1
                        for j in range(LQ // JQ):
                            j0 = j * JQ
                            pA = pso.tile([VW, JQ], F32, tag="pso", name=f"pA{c}_{j}")
                            pB = pso.tile([VW, JQ], F32, tag="pso", name=f"pB{c}_{j}")
                            for lk in range(NS):
                                stA = pss.tile([128, JQ], F32, tag="pss",
                                               name=f"stA{c}_{j}_{lk}")
                                stB = pss.tile([128, JQ], F32, tag="pss",
                                               name=f"stB{c}_{j}_{lk}")
                                for qh in range(JQ // 512):
                                    q0 = j0 + qh * 512
                                    nc.tensor.matmul(
                                        stA[:, qh * 512:(qh + 1) * 512],
                                        kht[c][0:64, lk * 128:(lk + 1) * 128],
                                        qht[c][0:64, q0:q0 + 512])
                                    nc.tensor.matmul(
                                        stB[:, qh * 512:(qh + 1) * 512],
                                        kht[c][64:128, lk * 128:(lk + 1) * 128],
                                        qht[c][64:128, q0:q0 + 512])
                                eA = epool.tile([128, JQ], F32R, tag="e",
                                                name=f"eA{c}_{j}_{lk}")
                                nc.scalar.activation(eA[:], stA[:], EXP, scale=0.125)
                                eB = epool.tile([128, JQ], F32R, tag="e",
                                                name=f"eB{c}_{j}_{lk}")
                                nc.scalar.activation(eB[:], stB[:], EXP, scale=0.125)
                                for qh in range(JQ // 512):
                                    s0 = qh * 512
                                    nc.tensor.matmul(
                                        pA[:, s0:s0 + 512],
                                        vh[lk][:, hA * VW:(hA + 1) * VW],
                                        eA[:, s0:s0 + 512],
                                        start=(lk == 0), stop=(lk == NS - 1))
                                    nc.tensor.matmul(
                                        pB[:, s0:s0 + 512],
                                        vh[lk][:, hB * VW:(hB + 1) * VW],
                                        eB[:, s0:s0 + 512],
                                        start=(lk == 0), stop=(lk == NS - 1))
                            for hh, px in ((hA, pA), (hB, pB)):
                                import concourse.bass as bass
                                drow = smalls.tile([1, JQ], F32, tag="drow",
                                                   name=f"d{hh}_{j}")
                                nc.vector.tensor_copy(out=drow[:], in_=px[DH:VW, :])
                                rrow = smalls.tile([1, JQ], F32, tag="rrow",
                                                   name=f"r{hh}_{j}")
                                nc.vector.reciprocal(rrow[:], drow[:])
                                bi = (hh * (LQ // JQ)) + j
                                nc.sync.dma_start(dbc[bi:bi + 1, :], rrow[:])
                                rbc = rbcp.tile([64, JQ], F32, tag="rbc",
                                                name=f"rb{hh}_{j}")
                                bc_src = bass.AP(tensor=dbc.tensor,
                                                 offset=bi * JQ,
                                                 ap=[[0, 64], [1, JQ]])
                                nc.sync.dma_start(rbc[:], bc_src)
                                r0 = (hh % 2) * 64
                                nc.vector.tensor_mul(
                                    oht[c][r0:r0 + 64, j0:j0 + JQ],
                                    px[0:DH, :], rbc[:])

                # ================= phase C: output projection =================
                with (
                    tc.tile_pool(name="wo", bufs=1) as wop,
                    tc.tile_pool(name="outp", bufs=4) as outp,
                    tc.tile_pool(name="psC", bufs=2, space="PSUM") as psC,
                ):
                    wo = []
                    for kc in range(HD_LOC // 128):
                        w = wop.tile([128, D], F32R, tag=f"wo{kc}", name=f"wo{kc}")
                        nc.sync.dma_start(w[:], woT[kc * 128:(kc + 1) * 128, :])
                        wo.append(w)
                    for m in range(D // 128):
                        for n in range(LQ // 512):
                            p = psC.tile([128, 512], F32, tag="psC", name=f"pc{m}_{n}")
                            for kc in range(HD_LOC // 128):
                                nc.tensor.matmul(
                                    p[:], wo[kc][:, m * 128:(m + 1) * 128],
                                    oht[kc][:, n * 512:(n + 1) * 512],
                                    start=(kc == 0), stop=(kc == HD_LOC // 128 - 1))
                            om = outp.tile([128, 512], F32, tag="om",
                                           name=f"om{m}_{n}")
                            nc.vector.tensor_copy(out=om[:], in_=p[:])
                            nc.sync.dma_start(
                                outT[m * 128:(m + 1) * 128,
                                     n * 512:(n + 1) * 512], om[:])

            if iters == 1:
                body()
            else:
                with tc.For_i(0, iters, 1):
                    body()

    nc.compile()
    return nc


def get_program(iters=1):
    if iters not in _PROG_CACHE:
        _PROG_CACHE[iters] = build_program(iters)
    return _PROG_CACHE[iters]


def shard_inputs(q, k, v, Wq, Wk, Wv, Wo):
    """Build the 8 per-core input maps (host-side layout prep only)."""
    q, k, v = (np.asarray(x, np.float32) for x in (q, k, v))
    Wq, Wk, Wv, Wo = (np.asarray(x, np.float32) for x in (Wq, Wk, Wv, Wo))
    in_maps = []
    for core in range(N_CORES):
        b, g = core // 2, core % 2
        rows = slice(g * HD_LOC, (g + 1) * HD_LOC)
        in_maps.append({
            "qT": np.ascontiguousarray(q[b].T),
            "kT": np.ascontiguousarray(k[b].T),
            "vT": np.ascontiguousarray(v[b].T),
            "wqT": np.ascontiguousarray(Wq[rows, :].T),
            "wkT": np.ascontiguousarray(Wk[rows, :].T),
            "wvT": np.ascontiguousarray(Wv[rows, :].T),
            "woT": np.ascontiguousarray(Wo[:, rows].T),
        })
    return in_maps


def gather_outputs(results):
    out = np.empty((B, LQ, D), np.float32)
    for b in range(B):
        acc = results[2 * b]["outT"] + results[2 * b + 1]["outT"]
        out[b] = acc.T
    return out


def kernel(q, k, v, Wq, Wk, Wv, Wo):
    from concourse.bass_utils import run_bass_kernel_spmd

    nc = get_program(1)
    in_maps = shard_inputs(q, k, v, Wq, Wk, Wv, Wo)
    res = run_bass_kernel_spmd(nc, in_maps, core_ids=list(range(N_CORES)))
    return gather_outputs(res.results)


# revision 30
# speedup vs baseline: 19.9869x; 19.9869x over previous
"""Trainium2 Bass kernel for 16-head MHA (B=4, L=2048, D=1024, fp32).

Sharding: batch x head-group over 8 cores. Core c handles batch c//2 and
heads (c%2)*8 .. (c%2)*8+7 (Megatron column-parallel QKV, row-parallel Wo).
Each core computes a partial output projection; the host sums the two
partials per batch.

Per-core device program (SPMD, identical on all cores):
  upfront: VH = v @ Wv_g.T  -> [2048, 8*65] (65th col of each head = 1.0 so
           the AV matmul also yields the softmax denominator as an extra
           output row); KHT = Wk_g @ k.T -> [512, 2048]; QHT column 0.
  main loop over 512-wide query columns j, head pairs c, key chunks lk:
      one [128,1024] score tile holds head A in cols 0:512 and head B in
      cols 512:1024 (two K=64 fp32r matmuls packed in PE row groups 0-1 /
      2-3, running concurrently); a single FD=1024 exp converts it to
      attention weights; two K=128 matmuls accumulate O_un (+ denominator
      row) per head. AV trails scores/exp by 2 steps so the PE never waits
      on ACT in program order. The Q projection for column j+1 and the
      output projection for column j-1 are drip-fed between steps to fill
      the PE's slack under the ACT-bound exp stream.
      normalize: O = O_un * (1/denom), denom broadcast across partitions
      via a DRAM-bounce DMA.

All matmul operands are float32r (full-rate fp32 mode on the PE array at
K=128; ~1e-4 relative error at K=1024).
"""

import sys

if "/opt/trn_rl_repo" not in sys.path:
    sys.path.insert(0, "/opt/trn_rl_repo")

import numpy as np

B, LQ, LV, D, H = 4, 2048, 2048, 1024, 16
DH = D // H            # 64
N_CORES = 8
H_LOC = H // 2         # 8 heads per core
HD_LOC = H_LOC * DH    # 512 head-dims per core
NKC = D // 128         # 8 contraction chunks for projections
NS = LV // 128         # 16 key chunks
NMC = HD_LOC // 128    # 4 head-dim chunks (head pairs)
JQ = 512               # query block width in attention
NJ = LQ // JQ          # 4 query columns
VW = DH + 1            # 65: per-head V width incl. ones column

_PROG_CACHE = {}


def build_program(iters=1, phases="abc"):
    import concourse.bass as bass
    import concourse.tile as tile
    from concourse import bacc, mybir

    F32 = mybir.dt.float32
    F32R = mybir.dt.float32r
    EXP = mybir.ActivationFunctionType.Exp

    nc = bacc.Bacc("TRN2", target_bir_lowering=False, debug=False,
                   num_devices=N_CORES)

    qT = nc.dram_tensor("qT", [D, LQ], F32R, kind="ExternalInput").ap()
    kT = nc.dram_tensor("kT", [D, LV], F32R, kind="ExternalInput").ap()
    vT = nc.dram_tensor("vT", [D, LV], F32R, kind="ExternalInput").ap()
    wqT = nc.dram_tensor("wqT", [D, HD_LOC], F32R, kind="ExternalInput").ap()
    wkT = nc.dram_tensor("wkT", [D, HD_LOC], F32R, kind="ExternalInput").ap()
    wvT = nc.dram_tensor("wvT", [D, HD_LOC], F32R, kind="ExternalInput").ap()
    woT = nc.dram_tensor("woT", [HD_LOC, D], F32R, kind="ExternalInput").ap()
    outT = nc.dram_tensor("outT", [D, LQ], F32, kind="ExternalOutput").ap()
    # DRAM bounce rows for broadcasting softmax 1/denom across partitions
    dbc = nc.dram_tensor("dbc", [2 * NMC * NJ, JQ], F32).ap()

    with tile.TileContext(nc) as tc:
        with (
            tc.tile_pool(name="persist", bufs=1) as persist,
            tc.tile_pool(name="wq", bufs=1) as wqp,
            tc.tile_pool(name="qact", bufs=12) as qactp,
            tc.tile_pool(name="qhtj", bufs=8) as qhtp,
            tc.tile_pool(name="ohtj", bufs=8) as ohtp,
            tc.tile_pool(name="e", bufs=4) as epool,
            tc.tile_pool(name="smalls", bufs=2) as smalls,
            tc.tile_pool(name="rbcp", bufs=2) as rbcp,
            tc.tile_pool(name="wo", bufs=1) as wop,
            tc.tile_pool(name="outp", bufs=2) as outp,
            tc.tile_pool(name="pss", bufs=2, space="PSUM") as pss,
            tc.tile_pool(name="pso", bufs=4, space="PSUM") as pso,
        ):
            def body():
                kht = [persist.tile([128, LV], F32R, tag=f"kht{m}", name=f"kht{m}")
                       for m in range(NMC)]
                vh = [persist.tile([128, H_LOC * VW], F32R, tag=f"vh{s}", name=f"vh{s}")
                      for s in range(NS)]
                ones8 = persist.tile([128, H_LOC], F32, tag="ones8", name="ones8")
                nc.vector.memset(ones8[:], 1.0)

                wq = []
                for kc in range(NKC):
                    w = wqp.tile([128, HD_LOC], F32R, tag=f"wq{kc}", name=f"wq{kc}")
                    nc.sync.dma_start(w[:], wqT[kc * 128:(kc + 1) * 128, :])
                    wq.append(w)
                wo = []
                for kc in range(HD_LOC // 128):
                    w = wop.tile([128, D], F32R, tag=f"wo{kc}", name=f"wo{kc}")
                    nc.sync.dma_start(w[:], woT[kc * 128:(kc + 1) * 128, :])
                    wo.append(w)
                qht = {}   # (m, n) -> [128, JQ] tile
                oht = {}   # (c, j) -> [128, JQ] tile

                def load_qact(n):
                    xs = []
                    for kc in range(NKC):
                        t = qactp.tile([128, 512], F32R, tag="qact",
                                       name=f"qx{n}_{kc}")
                        nc.sync.dma_start(
                            t[:], qT[kc * 128:(kc + 1) * 128,
                                     n * 512:(n + 1) * 512])
                        xs.append(t)
                    return xs

                def qproj_chain_mms(xs, n, m):
                    # one closure per matmul so chains can be woven one MM
                    # per attention step without starving the ACT stream
                    state = {}

                    def first():
                        state["p"] = pso.tile([128, 512], F32, tag="pso",
                                               name=f"qp{n}_{m}")
                    mms = []
                    for kc in range(NKC):
                        def mm(kc=kc):
                            if kc == 0:
                                first()
                            nc.tensor.matmul(
                                state["p"][:],
                                wq[kc][:, m * 128:(m + 1) * 128], xs[kc][:],
                                start=(kc == 0), stop=(kc == NKC - 1))
                            if kc == NKC - 1:
                                d = qhtp.tile([128, JQ], F32R, tag="qhtj",
                                              name=f"qh{n}_{m}")
                                nc.vector.tensor_copy(out=d[:], in_=state["p"][:])
                                qht[(m, n)] = d
                        mms.append(mm)
                    return mms

                def outproj_chain_mms(j, m):
                    state = {}
                    NWO = HD_LOC // 128
                    mms = []
                    for kc in range(NWO):
                        def mm(kc=kc):
                            if kc == 0:
                                state["p"] = pso.tile([128, 512], F32,
                                                       tag="pso",
                                                       name=f"cp{j}_{m}")
                            nc.tensor.matmul(
                                state["p"][:],
                                wo[kc][:, m * 128:(m + 1) * 128],
                                oht[(kc, j)][:],
                                start=(kc == 0), stop=(kc == NWO - 1))
                            if kc == NWO - 1:
                                om = outp.tile([128, JQ], F32, tag="om",
                                               name=f"om{j}_{m}")
                                nc.vector.tensor_copy(out=om[:], in_=state["p"][:])
                                nc.sync.dma_start(
                                    outT[m * 128:(m + 1) * 128,
                                         j * JQ:(j + 1) * JQ], om[:])
                                if m == D // 128 - 1:
                                    for c in range(NMC):
                                        oht.pop((c, j))
                        mms.append(mm)
                    return mms

                # ---------- upfront: V projection, K projection, Q col 0 ----
                with tc.tile_pool(name="wproj", bufs=1) as wpool:
                    actp = qactp
                    psA = pso
                    wv = []
                    for kc in range(NKC):
                        w = wpool.tile([128, HD_LOC], F32R, tag=f"w{kc}", name=f"wv{kc}")
                        nc.sync.dma_start(w[:], wvT[kc * 128:(kc + 1) * 128, :])
                        wv.append(w)
                    for sg in range(NS // 4):
                        vts = []
                        for kc in range(NKC):
                            t = actp.tile([128, 512], F32R, tag="qact", name=f"vt{sg}_{kc}")
                            nc.sync.dma_start(
                                t[:], vT[kc * 128:(kc + 1) * 128,
                                         sg * 512:(sg + 1) * 512])
                            vts.append(t)
                        for si in range(4):
                            s = sg * 4 + si
                            p = psA.tile([128, HD_LOC], F32, tag="pso", name=f"pv{s}")
                            for kc in range(NKC):
                                nc.tensor.matmul(
                                    p[:], vts[kc][:, si * 128:(si + 1) * 128],
                                    wv[kc][:], start=(kc == 0), stop=(kc == NKC - 1))
                            v3 = vh[s].rearrange("p (h e) -> p h e", e=VW)
                            nc.vector.tensor_copy(
                                out=v3[:, :, DH:VW],
                                in_=ones8.rearrange("p (h o) -> p h o", o=1))
                            nc.vector.tensor_copy(
                                out=v3[:, :, 0:DH],
                                in_=p.rearrange("p (h e) -> p h e", e=DH))

                    # K projection (uses the same act/weight slots as V)
                    ws = []
                    for kc in range(NKC):
                        w = wpool.tile([128, HD_LOC], F32R, tag=f"w{kc}",
                                       name=f"kw{kc}")
                        nc.sync.dma_start(w[:], wkT[kc * 128:(kc + 1) * 128, :])
                        ws.append(w)
                    for n in range(LQ // 512):
                        xs = []
                        for kc in range(NKC):
                            t = actp.tile([128, 512], F32R, tag="qact",
                                          name=f"kx{n}_{kc}")
                            nc.sync.dma_start(
                                t[:], kT[kc * 128:(kc + 1) * 128,
                                         n * 512:(n + 1) * 512])
                            xs.append(t)
                        for m in range(NMC):
                            p = psA.tile([128, 512], F32, tag="pso",
                                         name=f"kp{n}_{m}")
                            for kc in range(NKC):
                                nc.tensor.matmul(
                                    p[:], ws[kc][:, m * 128:(m + 1) * 128],
                                    xs[kc][:],
                                    start=(kc == 0), stop=(kc == NKC - 1))
                            nc.vector.tensor_copy(
                                out=kht[m][:, n * 512:(n + 1) * 512], in_=p[:])

                    # Q projection for column 0 (psA pool, B not running yet)
                    xs0 = load_qact(0)
                    for m in range(NMC):
                        p = psA.tile([128, 512], F32, tag="pso", name=f"qp0_{m}")
                        for kc in range(NKC):
                            nc.tensor.matmul(
                                p[:], wq[kc][:, m * 128:(m + 1) * 128], xs0[kc][:],
                                start=(kc == 0), stop=(kc == NKC - 1))
                        d = qhtp.tile([128, JQ], F32R, tag="qhtj", name=f"qh0_{m}")
                        nc.vector.tensor_copy(out=d[:], in_=p[:])
                        qht[(m, 0)] = d

                if "b" not in phases:
                    for m in range(NMC):
                        nc.sync.dma_start(outT[m * 128:(m + 1) * 128, :],
                                          kht[m][:].bitcast(F32))
                    nc.sync.dma_start(outT[0:128, 0:H_LOC * VW],
                                      vh[0][:].bitcast(F32))
                    return

                # ---------- main loop ----------
                PIPE = 2
                blocks = [(c, j) for j in range(NJ) for c in range(NMC)]
                steps = [(bi, lk) for bi in range(len(blocks))
                         for lk in range(NS)]
                psx = {}
                ets = {}
                qxs = {0: None}

                def emit_scores(bi, lk):
                    c, j = blocks[bi]
                    st = pss.tile([128, 2 * JQ], F32, tag="pss",
                                  name=f"st{bi}_{lk}")
                    nc.tensor.matmul(
                        st[:, 0:JQ],
                        kht[c][0:64, lk * 128:(lk + 1) * 128],
                        qht[(c, j)][0:64, :])
                    nc.tensor.matmul(
                        st[:, JQ:2 * JQ],
                        kht[c][64:128, lk * 128:(lk + 1) * 128],
                        qht[(c, j)][64:128, :])
                    e = epool.tile([128, 2 * JQ], F32R, tag="e",
                                   name=f"e{bi}_{lk}")
                    nc.scalar.activation(e[:], st[:], EXP, scale=0.125)
                    ets[(bi, lk)] = e

                def emit_av(bi, lk):
                    c, j = blocks[bi]
                    hA, hB = 2 * c, 2 * c + 1
                    if lk == 0:
                        psx[bi] = (
                            pso.tile([VW, JQ], F32, tag="pso", name=f"pa{bi}"),
                            pso.tile([VW, JQ], F32, tag="pso", name=f"pb{bi}"),
                        )
                    e = ets.pop((bi, lk))
                    pA, pB = psx[bi]
                    nc.tensor.matmul(
                        pA[:], vh[lk][:, hA * VW:(hA + 1) * VW], e[:, 0:JQ],
                        start=(lk == 0), stop=(lk == NS - 1))
                    nc.tensor.matmul(
                        pB[:], vh[lk][:, hB * VW:(hB + 1) * VW], e[:, JQ:2 * JQ],
                        start=(lk == 0), stop=(lk == NS - 1))
                    if lk == NS - 1:
                        emit_finalize(bi)

                def emit_finalize(bi):
                    c, j = blocks[bi]
                    oh = ohtp.tile([128, JQ], F32R, tag="ohtj", name=f"oh{bi}")
                    oht[(c, j)] = oh
                    for hi, px in enumerate(psx.pop(bi)):
                        oun = rbcp.tile([VW, JQ], F32, tag="oun",
                                        name=f"ou{bi}_{hi}")
                        nc.vector.tensor_copy(out=oun[:], in_=px[:])
                        rrow = smalls.tile([1, JQ], F32, tag="rrow",
                                           name=f"r{bi}_{hi}")
                        nc.vector.reciprocal(rrow[:], oun[DH:VW, :])
                        di = bi * 2 + hi
                        nc.sync.dma_start(dbc[di:di + 1, :], rrow[:])
                        rbc = rbcp.tile([64, JQ], F32, tag="rbc",
                                        name=f"rb{bi}_{hi}")
                        bc_src = bass.AP(tensor=dbc.tensor, offset=di * JQ,
                                         ap=[[0, 64], [1, JQ]])
                        nc.sync.dma_start(rbc[:], bc_src)
                        r0 = hi * 64
                        nc.vector.tensor_mul(
                            oh[r0:r0 + 64, :], oun[0:DH, :], rbc[:])

                do_c = "c" in phases
                chain_q = []   # woven chain matmuls, one per step
                for t in range(len(steps) + PIPE):
                    if t < len(steps):
                        emit_scores(*steps[t])
                    if t >= PIPE:
                        bi, lk = steps[t - PIPE]
                        c, j = blocks[bi]
                        emit_av(bi, lk)
                        sl = (bi % NMC) * NS + lk   # step within column
                        if sl == 0 and j < NJ - 1:
                            qxs[j + 1] = load_qact(j + 1)
                        if sl % NS == NS // 2 and j < NJ - 1:
                            for mm in qproj_chain_mms(qxs[j + 1], j + 1,
                                                      sl // NS):
                                mm()
                        if do_c and sl % (NS // 2) == 4 and j >= 1:
                            for mm in outproj_chain_mms(j - 1, sl // (NS // 2)):
                                mm()
                if do_c:
                    for m in range(D // 128):
                        for mm in outproj_chain_mms(NJ - 1, m):
                            mm()
                else:
                    for (c, j), oh in sorted(oht.items()):
                        nc.sync.dma_start(
                            outT[c * 128:(c + 1) * 128, j * JQ:(j + 1) * JQ],
                            oh[:].bitcast(F32))

            if iters == 1:
                body()
            else:
                with tc.For_i(0, iters, 1):
                    body()

    nc.compile()
    return nc


def get_program(iters=1, phases="abc"):
    key = (iters, phases)
    if key not in _PROG_CACHE:
        _PROG_CACHE[key] = build_program(iters, phases)
    return _PROG_CACHE[key]


def shard_inputs(q, k, v, Wq, Wk, Wv, Wo):
    """Build the 8 per-core input maps (host-side layout prep only)."""
    q, k, v = (np.asarray(x, np.float32) for x in (q, k, v))
    Wq, Wk, Wv, Wo = (np.asarray(x, np.float32) for x in (Wq, Wk, Wv, Wo))
    in_maps = []
    for core in range(N_CORES):
        b, g = core // 2, core % 2
        rows = slice(g * HD_LOC, (g + 1) * HD_LOC)
        in_maps.append({
            "qT": np.ascontiguousarray(q[b].T),
            "kT": np.ascontiguousarray(k[b].T),
            "vT": np.ascontiguousarray(v[b].T),
            "wqT": np.ascontiguousarray(Wq[rows, :].T),
            "wkT": np.ascontiguousarray(Wk[rows, :].T),
            "wvT": np.ascontiguousarray(Wv[rows, :].T),
            "woT": np.ascontiguousarray(Wo[:, rows].T),
        })
    return in_maps


def gather_outputs(results):
    out = np.empty((B, LQ, D), np.float32)
    for b in range(B):
        acc = results[2 * b]["outT"] + results[2 * b + 1]["outT"]
        out[b] = acc.T
    return out


def kernel(q, k, v, Wq, Wk, Wv, Wo):
    from concourse.bass_utils import run_bass_kernel_spmd

    nc = get_program(1)
    in_maps = shard_inputs(q, k, v, Wq, Wk, Wv, Wo)
    res = run_bass_kernel_spmd(nc, in_maps, core_ids=list(range(N_CORES)))
    return gather_outputs(res.results)
